# revision 9
# baseline (speedup 1.0000x reference)
"""Trainium2 Bass kernel for nn_MicroAdder (tiny dense transformer).

v4 — y-groups folded into mm2 (6-rows-per-t, two half-t matmuls per block),
interleaved emission so chain ACT/DVE ops head the engine queues, f16 chain.

Decomposition: per-element quantities are affine in the basis
[cos(ang), sin(ang), 1] per t. mm1 (per 128-row block) produces 6 linear
forms (q0, q1, a, e0, e1, r); a short f16 chain produces inv3 and z;
mm2 expands to (T, V) logits with lint rows
(z0*inv3, z1*inv3, u*inv3, w*inv3, inv3, a*inv3) per t — the head-projection
y contribution lives entirely in mm2's constant table, so no p-chain.

mm2 split: 6 rows x 34 t = 204 > 128 contraction limit, so each block runs
two matmuls over t-halves (6 x 17 = 102 rows each), writing adjacent column
ranges [0:238] / [238:476] of the same PSUM tile (output stays contiguous).

Sharding: pure data parallel over the batch dim across 8 NeuronCores.
"""

import math
import sys

import numpy as np

for _p in ("/opt/trn_rl_repo", "/root/.axon_site/_ro/trn_rl_repo"):
    if _p not in sys.path:
        sys.path.append(_p)

import ml_dtypes  # noqa: E402

import concourse.bacc as bacc  # noqa: E402
import concourse.bass as bass  # noqa: E402
import concourse.tile as tile  # noqa: E402
from concourse import mybir  # noqa: E402
from concourse.bass_utils import run_bass_kernel_spmd  # noqa: E402

# ---------------------------------------------------------------- problem dims
B, T, V = 65536, 34, 14
D, EPS, MAX_DIGITS = 5, 1e-5, 10
NCORES = 8
BC = B // NCORES            # rows per core = 8192
P = 128                     # partitions
NPER = BC // P              # rows per partition = 64
NBLK = NPER                 # blocks per core = 64
SGB = 16                    # blocks per supergroup
NSG = NBLK // SGB           # 4 supergroups
KB = 128                    # padded basis rows (69 used)
NG = 6                      # matmul1 groups
N1 = NG * T                 # 204
N2 = T * V                  # 476
TH = 17                     # t-half size
NL = 6                      # lint quantities per t
KL = NL * TH                # lint rows per half = 102
NH = TH * V                 # output columns per half = 238
NPRM = 8
FW = T * SGB                # 544 columns per supergroup

F32 = mybir.dt.float32
F16 = mybir.dt.float16
U8 = mybir.dt.uint8
AF = mybir.ActivationFunctionType
ALU = mybir.AluOpType
F16NP = np.float16

# group order in matmul1 output columns (g*T..g*T+T); Q first so the non-Q
# tail [2T:6T] is one contiguous copy
G_Q0, G_Q1, G_A, G_E0, G_E1, G_R = range(6)
NQ_A, NQ_E0, NQ_E1, NQ_R = range(4)
NQW = 4 * T                 # nonq tile width per block = 136

# lint column order within each 128-col half (units of TH)
L_Z0, L_Z1, L_U, L_W, L_I, L_A = range(NL)

# PRM slots (activation bias/scale vectors; chain scalars are baked)
(P_SSC, P_SB, P_CSC, P_CB, P_ZERO) = range(5)


# ---------------------------------------------------------------- host tables
def host_tables(tok_A, tok_start, tok_stride, sp_amp, sp_phase, sp_slope, sp_offset,
                norm_w, q_w, q_phase, out_A, out_B, fc1_w, fc2_w, head_w):
    f = np.float64
    A = f(tok_A)
    t = np.arange(T, dtype=f)
    th = 2.0 * np.pi * t / MAX_DIGITS + f(sp_phase)
    pos = np.stack([f(sp_amp) * np.cos(th), f(sp_amp) * np.sin(th),
                    f(sp_slope) * t + f(sp_offset)], axis=-1)
    k = pos @ np.asarray(q_w, f).T
    c0, s0 = np.cos(f(q_phase[0])), np.sin(f(q_phase[0]))
    q = k.copy()
    q[:, 0] = c0 * k[:, 0] - s0 * k[:, 1]
    q[:, 1] = s0 * k[:, 0] + c0 * k[:, 1]
    scores = (q @ k.T) / np.sqrt(f(5.0))
    sm = np.where(np.tril(np.ones((T, T), bool)), scores, -np.inf)
    sm = sm - sm.max(-1, keepdims=True)
    e = np.exp(sm)
    attn = e / e.sum(-1, keepdims=True)

    nw = np.asarray(norm_w, f)
    oA = np.asarray(out_A, f)[:, 0]
    oB = np.asarray(out_B, f)[0]
    S_t = A * A + (pos ** 2).sum(-1)
    rms1 = np.sqrt(S_t / D + EPS)

    M0 = attn * (A * nw[0] * oA[0] / rms1)[None, :]
    M1 = attn * (A * nw[1] * oA[1] / rms1)[None, :]
    c_t = attn @ ((pos * (nw[2:] * oA[2:])[None, :]).sum(-1) / rms1)

    g0 = np.asarray(fc2_w, f)[:, 0]
    g1 = np.asarray(fc2_w, f)[:, 1]
    projs = {
        G_Q0: nw * np.asarray(fc1_w, f)[0],
        G_Q1: nw * np.asarray(fc1_w, f)[1],
        G_E0: 2.0 * g0,
        G_E1: 2.0 * g1,
    }
    # R in the basis (u = cos(ang), w = sin(ang), 1); row 2T is the constant.
    R = np.zeros((KB, NG * T), dtype=f)
    dd = np.eye(T, dtype=f)
    for gi in range(NG):
        cols = slice(gi * T, (gi + 1) * T)
        if gi == G_A:
            R[0:T, cols] = M0.T
            R[T:2 * T, cols] = M1.T
            R[2 * T, cols] = c_t
        elif gi == G_R:
            b2 = (oB ** 2).sum()
            R[0:T, cols] = 2 * A * oB[0] * dd + b2 * M0.T
            R[T:2 * T, cols] = 2 * A * oB[1] * dd + b2 * M1.T
            R[2 * T, cols] = 2 * (pos * oB[None, 2:]).sum(-1) + b2 * c_t
        else:
            v = projs[gi]
            bv = (oB * v).sum()
            R[0:T, cols] = A * v[0] * dd + bv * M0.T
            R[T:2 * T, cols] = A * v[1] * dd + bv * M1.T
            R[2 * T, cols] = (pos * v[None, 2:]).sum(-1) + bv * c_t

    # fold 1/D into the e-groups and r-group
    R[:, G_E0 * T:(G_E1 + 1) * T] *= 1.0 / D
    R[:, G_R * T:(G_R + 1) * T] *= 1.0 / D

    G00, G01, G11 = (g0 * g0).sum(), (g0 * g1).sum(), (g1 * g1).sum()
    if G00 > 1e-30:
        sq0, rat = np.sqrt(G00), G01 / G00
        c3 = np.sqrt(max(G11 - G01 * G01 / G00, 0.0))
    else:
        sq0, rat, c3 = 0.0, 0.0, np.sqrt(G11)
    sc05 = np.sqrt(1.0 / D)

    # head-projection vectors and mm2 tables
    hv0 = nw * np.asarray(head_w, f)[0]
    hv1 = nw * np.asarray(head_w, f)[1]
    H = np.array([[(g0 * hv0).sum(), (g0 * hv1).sum()],
                  [(g1 * hv0).sum(), (g1 * hv1).sum()]])
    bv0 = (oB * hv0).sum()
    bv1 = (oB * hv1).sum()

    dvoc = np.arange(V, dtype=f)
    ang = f(tok_start) + dvoc * f(tok_stride)
    E = np.stack([A * np.cos(ang), A * np.sin(ang)], axis=-1)  # (V, 2)

    # RHS2[half][row, col]: row = q*TH + (t-t0), col = (t-t0)*V + v
    RHS2 = np.zeros((KB, 2, NH), dtype=f)
    for half in range(2):
        t0 = half * TH
        for tt in range(TH):
            t_ = t0 + tt
            cols = slice(tt * V, (tt + 1) * V)
            RHS2[L_Z0 * TH + tt, half, cols] = H[0, 0] * E[:, 0] + H[0, 1] * E[:, 1]
            RHS2[L_Z1 * TH + tt, half, cols] = H[1, 0] * E[:, 0] + H[1, 1] * E[:, 1]
            RHS2[L_U * TH + tt, half, cols] = A * (hv0[0] * E[:, 0] + hv1[0] * E[:, 1])
            RHS2[L_W * TH + tt, half, cols] = A * (hv0[1] * E[:, 0] + hv1[1] * E[:, 1])
            RHS2[L_I * TH + tt, half, cols] = (
                (pos[t_] * hv0[2:]).sum() * E[:, 0]
                + (pos[t_] * hv1[2:]).sum() * E[:, 1])
            RHS2[L_A * TH + tt, half, cols] = bv0 * E[:, 0] + bv1 * E[:, 1]

    # S' = S/D + EPS, tiled per supergroup; shipped as f16
    SROW = np.tile(S_t / D + EPS, SGB)[None, :]

    PRM = np.zeros((1, NPRM), dtype=np.float32)
    PRM[0, P_SSC] = f(tok_stride)
    PRM[0, P_SB] = f(tok_start)
    PRM[0, P_CSC] = -f(tok_stride)
    PRM[0, P_CB] = np.pi / 2.0 - f(tok_start)
    PRM[0, P_ZERO] = 0.0
    sc = {"rat": float(rat), "sq0": float(sq0 * sc05), "c3": float(c3 * sc05)}
    return (np.ascontiguousarray(R.astype(F16NP)),
            np.ascontiguousarray(RHS2.reshape(KB, 2 * NH).astype(F16NP)),
            np.ascontiguousarray(SROW.astype(F16NP)),
            PRM, sc)


def _act_rsqrt(nc, out, in_):
    """ACT Rsqrt via direct InstActivation (wrapper bans it for accuracy;
    fine at this kernel's 2e-2 tolerance)."""
    eng = nc.scalar
    inputs = [eng.lower_ap(in_)]
    for arg in (0.0, 1.0, 0.0):  # bias, scale, alpha
        inputs.append(mybir.ImmediateValue(dtype=mybir.dt.float32, value=arg))
    return eng.add_instruction(
        mybir.InstActivation(
            name=eng.bass.get_next_instruction_name(),
            func=AF.Rsqrt,
            ins=inputs,
            outs=[eng.lower_ap(out)],
        )
    )


# ---------------------------------------------------------------- bass kernel
def build_bass(sc=None):
    """sc: chain scalars baked as compile-time immediates (AP-scalar
    TensorScalar ops hit a ~7us/op slow path on HW)."""
    if sc is None:
        sc = {"rat": 0.0, "sq0": 1.0, "c3": 1.0}
    nc = bacc.Bacc("TRN2", target_bir_lowering=False, debug=False)

    idx_d = nc.dram_tensor("idx", [BC, T], U8, kind="ExternalInput").ap()
    r_d = nc.dram_tensor("R", [KB, N1], F16, kind="ExternalInput").ap()
    rhs2_d = nc.dram_tensor("RHS2", [KB, 2 * NH], F16, kind="ExternalInput").ap()
    srow_d = nc.dram_tensor("SROW", [1, FW], F16, kind="ExternalInput").ap()
    prm_d = nc.dram_tensor("PRM", [1, NPRM], F32, kind="ExternalInput").ap()
    out_d = nc.dram_tensor("out", [BC, N2], F16, kind="ExternalOutput").ap()

    idx_v = idx_d.rearrange("(p n) t -> p n t", p=P)       # [128, 64, 34]
    out_v4 = out_d.rearrange("(p g f) c -> p g (f c)", p=P, f=4)  # [128,16,1904]

    with tile.TileContext(nc) as tc:
        with (
            tc.tile_pool(name="const", bufs=1) as cpool,
            tc.tile_pool(name="uwp", bufs=1) as uwpool,
            tc.tile_pool(name="uwt", bufs=2) as uwtp,
            tc.tile_pool(name="dr", bufs=2) as drp,
            tc.tile_pool(name="sg", bufs=2) as sgp,
            tc.tile_pool(name="lit", bufs=2) as litp,
            tc.tile_pool(name="outsb", bufs=3) as outp,
            tc.tile_pool(name="pmm1", bufs=2, space="PSUM") as pmm1p,
            tc.tile_pool(name="pout", bufs=2, space="PSUM") as poutp,
        ):
            # ---- constants
            r_sb = cpool.tile([KB, N1], F16)
            nc.scalar.dma_start(r_sb[:], r_d)
            rhs2_sb = cpool.tile([KB, 2, NH], F16)
            nc.scalar.dma_start(rhs2_sb[:].rearrange("k a b -> k (a b)"), rhs2_d)
            s_sb = cpool.tile([P, FW], F16)
            nc.scalar.dma_start(s_sb[:], srow_d.broadcast_to([P, FW]))
            prm_sb = cpool.tile([P, NPRM], F32)
            nc.scalar.dma_start(prm_sb[:], prm_d.broadcast_to([P, NPRM]))

            def prm(i):
                return prm_sb[:, i:i + 1]

            # ---------------- phase A: idx -> basis [cos, sin, 1] (f16);
            # ACT Sin reads the u8 ids directly (args within [-pi, pi])
            uw = uwpool.tile([P, NBLK, KB], F16)
            idx_t = uwpool.tile([P, NBLK * T], U8)
            nc.scalar.dma_start(idx_t[:], idx_v[:, :, :])
            idx3 = idx_t[:].rearrange("p (n t) -> p n t", t=T)
            nc.vector.memset(uw[:, :, 2 * T:2 * T + 1], 1.0)
            nc.vector.memset(uw[:, :, 2 * T + 1:KB], 0.0)

            def phase_a(sg):
                blk = slice(sg * SGB, (sg + 1) * SGB)
                nc.scalar.activation(uw[:, blk, 0:T], idx3[:, blk, :], AF.Sin,
                                     bias=prm(P_CB), scale=prm(P_CSC))
                nc.scalar.activation(uw[:, blk, T:2 * T], idx3[:, blk, :], AF.Sin,
                                     bias=prm(P_SB), scale=prm(P_SSC))

            def t1_transpose(sg):
                j0 = sg * SGB
                uwT = uwtp.tile([KB, SGB * P], F16, tag="uwT")
                uwT3 = uwT[:].rearrange("k (j m) -> k j m", m=P)
                nc.sync.dma_start(
                    uwT3, uw[:, j0:j0 + SGB, :].rearrange("p j k -> p (j k)"),
                    transpose=True)
                return uwT3

            def mm1_drains(sg, uwT3):
                """Per h (2 blocks): 2 matmuls, one ACT Relu (q groups), one
                copy (non-q groups, f16; engine alternates)."""
                rho = drp.tile([P, SGB, 2 * T], F16, tag="rho")
                nonq = drp.tile([P, SGB, NQW], F16, tag="nonq")
                for h in range(SGB // 2):
                    jj = 2 * h
                    pm = pmm1p.tile([P, 2, 512], F32, tag="pm")
                    for b in range(2):
                        nc.tensor.matmul(
                            pm[:, b, 0:N1],
                            uwT3[0:2 * T + 1, jj + b, :],
                            r_sb[0:2 * T + 1, :],
                            start=True, stop=True)
                    sl = slice(jj, jj + 2)
                    nc.scalar.activation(rho[:, sl, :], pm[:, :, 0:2 * T],
                                         AF.Relu, bias=0.0, scale=1.0)
                    if h % 2 == 0:
                        nc.vector.tensor_copy(nonq[:, sl, :],
                                              pm[:, :, 2 * T:NG * T])
                    else:
                        nc.scalar.copy(nonq[:, sl, :], pm[:, :, 2 * T:NG * T])
                return rho, nonq

            def nq(t, g):
                return t[:, :, g * T:(g + 1) * T]

            def chain(sg, dr, la, lb):
                """f16 elementwise chain -> lint halves (la: t 0..16,
                lb: t 17..33). Emitted FIRST in the iteration so its ACT/DVE
                ops head the in-order engine queues."""
                rho, nonq = dr
                rho0 = rho[:, :, 0:T]
                rho1 = rho[:, :, T:2 * T]

                tab = sgp.tile([P, SGB, 2 * T], F16, tag="tab")
                nc.vector.tensor_mul(tab[:],
                                     nonq[:, :, NQ_E0 * T:(NQ_E1 + 1) * T], rho)
                ar = sgp.tile([P, FW], F16, tag="ar")
                nc.vector.tensor_mul(ar[:].rearrange("p (n t) -> p n t", t=T),
                                     nq(nonq, NQ_A), nq(nonq, NQ_R))
                m2 = sgp.tile([P, FW], F16, tag="m2")
                nc.vector.tensor_add(m2[:], ar[:], s_sb[:])
                inv2 = sgp.tile([P, FW], F16, tag="inv2")
                _act_rsqrt(nc, inv2[:], m2[:])
                inv23 = inv2[:].rearrange("p (n t) -> p n t", t=T)

                tau = sgp.tile([P, FW], F16, tag="tau")
                nc.gpsimd.tensor_add(tau[:].rearrange("p (n t) -> p n t", t=T),
                                     tab[:, :, 0:T], tab[:, :, T:2 * T])

                z0 = sgp.tile([P, FW], F16, tag="z0")
                nc.vector.tensor_mul(z0[:].rearrange("p (n t) -> p n t", t=T),
                                     rho0, inv23)
                z1 = sgp.tile([P, FW], F16, tag="z1")
                nc.vector.tensor_mul(z1[:].rearrange("p (n t) -> p n t", t=T),
                                     rho1, inv23)
                it2 = sgp.tile([P, FW], F16, tag="it2")
                nc.vector.tensor_mul(it2[:], tau[:], inv2[:])

                z1r = sgp.tile([P, FW], F16, tag="z1r")
                nc.vector.tensor_scalar_mul(z1r[:], z1[:], sc["rat"])
                v1 = sgp.tile([P, FW], F16, tag="v1")
                nc.vector.tensor_add(v1[:], z1r[:], z0[:])
                v1sq = sgp.tile([P, FW], F16, tag="v1sq")
                nc.scalar.activation(v1sq[:], v1[:], AF.Square,
                                     bias=prm(P_ZERO), scale=sc["sq0"])
                v2sq = sgp.tile([P, FW], F16, tag="v2sq")
                nc.scalar.activation(v2sq[:], z1[:], AF.Square,
                                     bias=prm(P_ZERO), scale=sc["c3"])

                m3a = sgp.tile([P, FW], F16, tag="m3a")
                nc.gpsimd.tensor_add(m3a[:], m2[:], it2[:])
                m3b = sgp.tile([P, FW], F16, tag="m3b")
                nc.vector.tensor_add(m3b[:], v1sq[:], v2sq[:])
                m3 = sgp.tile([P, FW], F16, tag="m3")
                nc.vector.tensor_add(m3[:], m3a[:], m3b[:])
                inv3 = sgp.tile([P, FW], F16, tag="inv3")
                _act_rsqrt(nc, inv3[:], m3[:])
                inv33 = inv3[:].rearrange("p (n t) -> p n t", t=T)

                # lint writes: per half, 6 quantities x [P, 16, 17]
                z03 = z0[:].rearrange("p (n t) -> p n t", t=T)
                z13 = z1[:].rearrange("p (n t) -> p n t", t=T)
                blk = slice(sg * SGB, (sg + 1) * SGB)
                for half, lt in ((0, la), (1, lb)):
                    ts = slice(half * TH, half * TH + TH)
                    iv = inv33[:, :, ts]

                    def lcol(q):
                        return lt[:, :, q * TH:(q + 1) * TH]

                    nc.vector.tensor_mul(lcol(L_Z0), z03[:, :, ts], iv)
                    nc.vector.tensor_mul(lcol(L_Z1), z13[:, :, ts], iv)
                    nc.vector.tensor_mul(lcol(L_U),
                                         uw[:, blk, half * TH:half * TH + TH], iv)
                    nc.gpsimd.tensor_mul(
                        lcol(L_W), uw[:, blk, T + half * TH:T + half * TH + TH],
                        iv)
                    nc.gpsimd.tensor_copy(lcol(L_I), iv)
                    nc.gpsimd.tensor_mul(lcol(L_A),
                                         nq(nonq, NQ_A)[:, :, ts], iv)

            def t2_mm2_store(sg, la, lb, laT3, lbT3):
                nc.sync.dma_start(
                    laT3, la[:].rearrange("p j k -> p (j k)"), transpose=True)
                nc.sync.dma_start(
                    lbT3, lb[:].rearrange("p j k -> p (j k)"), transpose=True)
                for q in range(SGB // 4):
                    o_sb = outp.tile([P, 4, N2], F16, tag="osb")
                    for hh in range(2):
                        jj = 4 * q + 2 * hh
                        po = poutp.tile([P, 2, 512], F32, tag="po")
                        for b in range(2):
                            nc.tensor.matmul(po[:, b, 0:NH],
                                             laT3[0:KL, jj + b, :],
                                             rhs2_sb[0:KL, 0, :],
                                             start=True, stop=True)
                            nc.tensor.matmul(po[:, b, NH:N2],
                                             lbT3[0:KL, jj + b, :],
                                             rhs2_sb[0:KL, 1, :],
                                             start=True, stop=True)
                        if (q + hh) % 2 == 0:
                            nc.scalar.copy(o_sb[:, 2 * hh:2 * hh + 2, :],
                                           po[:, :, 0:N2])
                        else:
                            nc.vector.tensor_copy(o_sb[:, 2 * hh:2 * hh + 2, :],
                                                  po[:, :, 0:N2])
                    nc.sync.dma_start(out_v4[:, sg * 4 + q, :],
                                      o_sb[:].rearrange("p f c -> p (f c)"))

            # persistent lint tiles (pad [KL:128] feeds only unread transpose
            # rows; zero once)
            lints = []
            for i in range(2):
                pair = []
                for hname in ("a", "b"):
                    lt = uwpool.tile([P, SGB, KB], F16, tag=f"lint{hname}{i}")
                    nc.vector.memset(lt[:, :, KL:KB], 0.0)
                    ltT = litp.tile([KB, SGB * P], F16, tag=f"lintT{hname}")
                    pair.append((lt, ltT[:].rearrange("k (j m) -> k j m", m=P)))
                lints.append(pair)

            # ---------------- driver
            uwTs = []
            for sg in range(NSG):
                phase_a(sg)
                uwTs.append(t1_transpose(sg))

            dr = mm1_drains(0, uwTs[0])
            for sg in range(NSG):
                (la, laT3), (lb, lbT3) = lints[sg % 2]
                chain(sg, dr, la, lb)
                dr = mm1_drains(sg + 1, uwTs[sg + 1]) if sg + 1 < NSG else None
                t2_mm2_store(sg, la, lb, laT3, lbT3)

    nc.compile()
    return nc


_CACHE = {}


def _get_nc(sc):
    key = tuple(sorted(sc.items()))
    if _CACHE.get("key") != key:
        _CACHE["nc"] = build_bass(sc)
        _CACHE["key"] = key
    return _CACHE["nc"]


def kernel(**inputs) -> np.ndarray:
    idx = np.asarray(inputs["idx"]).astype(np.uint8)
    kw = {k: np.asarray(v, np.float64) for k, v in inputs.items() if k != "idx"}
    R, RHS2, SROW, PRM, sc = host_tables(**kw)
    nc = _get_nc(sc)
    in_maps = [
        {"idx": idx[c * BC:(c + 1) * BC], "R": R, "RHS2": RHS2,
         "SROW": SROW, "PRM": PRM}
        for c in range(NCORES)
    ]
    res = run_bass_kernel_spmd(nc, in_maps, core_ids=list(range(NCORES)))
    out = np.concatenate([res.results[c]["out"] for c in range(NCORES)], axis=0)
    return np.ascontiguousarray(out.astype(np.float32).reshape(B, T, V))
